# revision 7
# baseline (speedup 1.0000x reference)
"""Trainium2 Bass kernel for MultiHeadAttention w/ local position embedding (LPE).

Reference computation (per batch b):
  qkv = x @ qkv_w.T -> q,k,v per head [N, 64]
  attn = softmax(q @ k.T / 8) ; out_h = attn @ v + depthwise3x3(v_spatial)
  y = concat_h(out_h) @ proj_w.T + proj_b

Sharding: 8 cores; core c handles batch b = c//4, heads h0..h0+2 where
h0 = (c%4)*3.  Each core returns a partial projection output for its 3
heads; the host sums the 4 partials per batch and adds proj_b once
(row-parallel gather).

On-device layout is "transposed world" — tensors keyed [feature, token] so
the PE contracts over partitions naturally:
  - xT [768, NT] streamed chunk-wise into SBUF (NT=3200 zero-padded tokens)
  - qT/kT computed duplicated on both partition halves (stationary [W|W])
    enabling row-tiled (2 concurrent) K=64 score matmuls
  - scores S^T [kv, q] accumulate in PSUM; exp on ACT (softmax scale fused,
    no max-subtraction needed: |s/8| <~ 7 for this distribution) -> f32r
  - PV via lhsT=[V | ones-col] (M=65): PSUM row 64 accumulates the softmax
    denominator l for free during the same rhs stream
  - per chunk: DVE reciprocal of the l row, PE outer-product (K=1 matmul)
    broadcasts 1/l across 64 partitions, DVE multiply normalizes straight
    into the per-head output slot
  - LPE depthwise 3x3 conv = 9 diagonal-matrix matmuls over a zero-padded
    [64, 66x50] spatial grid accumulated in PSUM, added into the output
  - final projection: 3 x K=64 k-tiles per token tile
All matmuls run in float32r (full PE rate at free-dim >=256, ~1.5e-4 rel
error), with fp32 PSUM accumulation.
"""

import numpy as np

B, N, D, H, DH = 2, 3073, 768, 12, 64
NT = 3200               # padded token count (25 * 128)
NKV = NT // 128         # 25 kv tiles
SCALE = 0.125
GH, GW = 64, 48         # spatial grid of the 3072 patch tokens
PGW = GW + 2            # 50
PGH = GH + 2            # 66
CHUNKS = [(i * 512, 512) for i in range(6)] + [(3072, 128)]

TRACE = False
LAST_RESULT = None

_CACHE = {}


def _build_nc():
    import concourse.bacc as bacc
    import concourse.mybir as mybir
    import concourse.tile as tile

    F32 = mybir.dt.float32
    F32R = mybir.dt.float32r
    EXP = mybir.ActivationFunctionType.Exp

    nc = bacc.Bacc()
    xt_d = nc.dram_tensor("xt", (6, 128, NT), F32R, kind="ExternalInput")
    wq_d = nc.dram_tensor("wq", (3, 6, 128, 128), F32R, kind="ExternalInput")
    wk_d = nc.dram_tensor("wk", (3, 6, 128, 128), F32R, kind="ExternalInput")
    wv_d = nc.dram_tensor("wv", (6, 128, 192), F32R, kind="ExternalInput")
    pj_d = nc.dram_tensor("projt", (192, 768), F32R, kind="ExternalInput")
    dcv_d = nc.dram_tensor("dconv", (64, 9, 64), F32R, kind="ExternalInput")
    cvb_d = nc.dram_tensor("convb", (64, 1), F32, kind="ExternalInput")
    idn_d = nc.dram_tensor("ident", (128, 128), F32R, kind="ExternalInput")
    vnc_d = nc.dram_tensor("vncol", (128, NKV, 1), F32R, kind="ExternalInput")
    one_d = nc.dram_tensor("ones", (65, 64), F32R, kind="ExternalInput")
    zp_d = nc.dram_tensor("zpad", (64, PGH * PGW), F32R, kind="ExternalInput")
    y_d = nc.dram_tensor("y", (NT, 768), F32, kind="ExternalOutput")

    with tile.TileContext(nc) as tc:
        with tc.tile_pool(name="big", bufs=1) as big, \
             tc.tile_pool(name="wts", bufs=1) as wts:
            qd = big.tile([128, 3, NT], F32R)     # qT dup'd on both halves
            kd = big.tile([128, 3, NT], F32R)     # kT dup'd on both halves
            vt = big.tile([128, 2, NT], F32R)     # vT packed: h0/h1 | h2
            vn = big.tile([128, NKV, 65], F32R)   # natural V + ones col
            opv = big.tile([64, 3, NT], F32R)     # normalized out^T per head

            wq_sb = wts.tile([128, 3, 6, 128], F32R)
            wk_sb = wts.tile([128, 3, 6, 128], F32R)
            wv_sb = wts.tile([128, 6, 192], F32R)
            pj_sb = wts.tile([64, 3, 768], F32R)
            dcv_sb = wts.tile([64, 9, 64], F32R)
            cvb_sb = wts.tile([64, 1], F32)
            idn_sb = wts.tile([128, 128], F32R)
            one_sb = wts.tile([65, 64], F32R)

            for h in range(3):
                for kt in range(6):
                    nc.sync.dma_start(out=wq_sb[:, h, kt, :], in_=wq_d[h, kt])
                    nc.sync.dma_start(out=wk_sb[:, h, kt, :], in_=wk_d[h, kt])
            for kt in range(6):
                nc.sync.dma_start(out=wv_sb[:, kt, :], in_=wv_d[kt])
            for hh in range(3):
                nc.sync.dma_start(out=pj_sb[:, hh, :],
                                  in_=pj_d[hh * 64:(hh + 1) * 64, :])
            nc.sync.dma_start(out=dcv_sb, in_=dcv_d[:, :, :])
            nc.sync.dma_start(out=cvb_sb, in_=cvb_d[:, :])
            nc.sync.dma_start(out=idn_sb, in_=idn_d[:, :])
            nc.sync.dma_start(out=one_sb, in_=one_d[:, :])
            nc.sync.dma_start(out=vn[:, :, 64:65], in_=vnc_d[:, :, :])

            # ---- Phase A: q/k/v projections, streaming xT chunk-wise ----
            with tc.tile_pool(name="xtc", bufs=2) as xtp, \
                 tc.tile_pool(name="psprj", bufs=2, space="PSUM") as psprj:
                for (c0, cw) in CHUNKS:
                    xtc = xtp.tile([128, 6, 512], F32R)
                    nc.sync.dma_start(
                        out=xtc[:, :, :cw],
                        in_=xt_d[:, :, c0:c0 + cw].rearrange("k p c -> p k c"))
                    for h in range(3):
                        psQ = psprj.tile([128, 512], F32, tag="q")
                        psK = psprj.tile([128, 512], F32, tag="k")
                        for kt in range(6):
                            nc.tensor.matmul(psQ[:, :cw], wq_sb[:, h, kt, :],
                                             xtc[:, kt, :cw],
                                             start=(kt == 0), stop=(kt == 5))
                        for kt in range(6):
                            nc.tensor.matmul(psK[:, :cw], wk_sb[:, h, kt, :],
                                             xtc[:, kt, :cw],
                                             start=(kt == 0), stop=(kt == 5))
                        nc.vector.tensor_copy(out=qd[:, h, c0:c0 + cw],
                                              in_=psQ[:, :cw])
                        nc.vector.tensor_copy(out=kd[:, h, c0:c0 + cw],
                                              in_=psK[:, :cw])
                    for vi in range(3):
                        psV = psprj.tile([64, 512], F32, tag="v")
                        for kt in range(6):
                            nc.tensor.matmul(
                                psV[:, :cw],
                                wv_sb[:, kt, vi * 64:(vi + 1) * 64],
                                xtc[:, kt, :cw],
                                start=(kt == 0), stop=(kt == 5))
                        dst = (vt[0:64, 0, c0:c0 + cw],
                               vt[64:128, 0, c0:c0 + cw],
                               vt[0:64, 1, c0:c0 + cw])[vi]
                        nc.vector.tensor_copy(out=dst, in_=psV[:, :cw])

            # ---- Phase B: per-head attention + LPE ----
            for h in range(3):
                vbase = (0, 64, 0)[h]
                vslot = (0, 0, 1)[h]
                vsrc = vt[vbase:vbase + 64, vslot, :]

                # natural V tiles via PE transpose (row-positioned for h1)
                with tc.tile_pool(name=f"pst{h}", bufs=2, space="PSUM") as pst:
                    for t in range(NKV):
                        ptr = pst.tile([128, 64], F32R, tag="tr")
                        nc.tensor.transpose(
                            ptr, vsrc[:, t * 128:(t + 1) * 128],
                            idn_sb[vbase:vbase + 64, vbase:vbase + 64],
                            tile_position=(vbase, 0))
                        nc.vector.tensor_copy(out=vn[:, t, 0:64], in_=ptr)

                # attention over q-chunks
                with tc.tile_pool(name=f"att{h}", bufs=3) as att, \
                     tc.tile_pool(name=f"nrm{h}", bufs=2) as nrm, \
                     tc.tile_pool(name=f"pss{h}", bufs=2, space="PSUM") as pss, \
                     tc.tile_pool(name=f"pso{h}", bufs=2, space="PSUM") as pso, \
                     tc.tile_pool(name=f"psr{h}", bufs=2, space="PSUM") as psr:
                    for (c0, cw) in CHUNKS:
                        psO = pso.tile([65, 512], F32, tag="o")
                        for jp in range(13):
                            kva, kvb = 2 * jp, 2 * jp + 1
                            npair = 2 if kvb < NKV else 1
                            psS = pss.tile([128, 2, 512], F32, tag="s")
                            nc.tensor.matmul(
                                psS[:, 0, :cw],
                                kd[0:64, h, kva * 128:(kva + 1) * 128],
                                qd[0:64, h, c0:c0 + cw],
                                start=True, stop=True, tile_position=(0, 0))
                            if npair == 2:
                                nc.tensor.matmul(
                                    psS[:, 1, :cw],
                                    kd[64:128, h, kvb * 128:(kvb + 1) * 128],
                                    qd[64:128, h, c0:c0 + cw],
                                    start=True, stop=True,
                                    tile_position=(64, 0))
                            es = att.tile([128, 2, 512], F32R, tag="es")
                            nc.scalar.activation(es[:, 0:npair, :cw],
                                                 psS[:, 0:npair, :cw],
                                                 EXP, scale=SCALE)
                            nc.tensor.matmul(psO[:, :cw], vn[:, kva, :],
                                             es[:, 0, :cw],
                                             start=(jp == 0), stop=(npair == 1))
                            if npair == 2:
                                nc.tensor.matmul(psO[:, :cw], vn[:, kvb, :],
                                                 es[:, 1, :cw],
                                                 start=False, stop=False)
                        # normalization: 1/l broadcast via PE outer product
                        rcp = nrm.tile([65, 512], F32R, tag="rcp")
                        with nc.allow_low_precision(reason="f32r 1/l (1.5e-4)"):
                            nc.vector.reciprocal(rcp[64:65, :cw],
                                                 psO[64:65, :cw])
                        psR = psr.tile([64, 512], F32, tag="r")
                        nc.tensor.matmul(psR[:, :cw], one_sb[64:65, :],
                                         rcp[64:65, :cw], start=True, stop=True,
                                         tile_position=(64, 0))
                        rb = nrm.tile([64, 512], F32R, tag="rb")
                        nc.vector.tensor_copy(out=rb[:, :cw], in_=psR[:, :cw])
                        nc.vector.tensor_mul(opv[:, h, c0:c0 + cw],
                                             psO[0:64, :cw], rb[:, :cw])

                # LPE depthwise conv, accumulated into opv
                with tc.tile_pool(name=f"lpe{h}", bufs=2) as lpp, \
                     tc.tile_pool(name=f"psl{h}", bufs=2, space="PSUM") as psl:
                    vpad = lpp.tile([64, PGH * PGW], F32R, tag="vpad")
                    nc.sync.dma_start(out=vpad, in_=zp_d[:, :])
                    gv = vpad.rearrange("p (r c) -> p r c", c=PGW)
                    nc.vector.tensor_copy(
                        out=gv[:, 1:1 + GH, 1:1 + GW],
                        in_=vsrc[:, 1:1 + GH * GW].rearrange(
                            "p (r c) -> p r c", c=GW))
                    for oc in range(8):
                        psL = psl.tile([64, 8 * GW], F32, tag="lpe")
                        for j in range(9):
                            dy, dx = j // 3, j % 3
                            rview = gv[:, oc * 8 + dy: oc * 8 + dy + 8,
                                       dx:dx + GW]
                            nc.tensor.matmul(psL, dcv_sb[:, j, :], rview,
                                             start=(j == 0), stop=(j == 8))
                        tmp = lpp.tile([64, 8 * GW], F32R, tag="tmp")
                        nc.vector.tensor_scalar_add(out=tmp, in0=psL,
                                                    scalar1=cvb_sb)
                        tok = slice(1 + oc * 8 * GW, 1 + (oc + 1) * 8 * GW)
                        nc.vector.tensor_add(opv[:, h, tok], opv[:, h, tok],
                                             tmp)

            # ---- Phase C: output projection ----
            with tc.tile_pool(name="ysb", bufs=3) as ysb, \
                 tc.tile_pool(name="psy", bufs=2, space="PSUM") as psy:
                for t in range(NKV):
                    tsl = slice(t * 128, (t + 1) * 128)
                    psY = psy.tile([128, 768], F32)
                    for (n0, nn) in [(0, 512), (512, 256)]:
                        for hh in range(3):
                            nc.tensor.matmul(psY[:, n0:n0 + nn],
                                             opv[:, hh, tsl],
                                             pj_sb[:, hh, n0:n0 + nn],
                                             start=(hh == 0), stop=(hh == 2))
                    yt = ysb.tile([128, 768], F32)
                    nc.vector.tensor_copy(out=yt, in_=psY)
                    nc.sync.dma_start(out=y_d[tsl, :], in_=yt)
    nc.compile()
    return nc


def _prep_inputs(x, qkv_w, proj_w, conv_w, conv_b):
    """Host-side sharding/layout prep. Returns list of 8 per-core input maps."""
    f32 = np.float32
    ident = np.eye(128, dtype=f32)
    conv9 = conv_w.reshape(64, 9).astype(f32)
    dconv = np.zeros((64, 9, 64), dtype=f32)
    ii = np.arange(64)
    for j in range(9):
        dconv[ii, j, ii] = conv9[:, j]
    convb = conv_b.reshape(64, 1).astype(f32)
    vncol = np.ones((128, NKV, 1), dtype=f32)
    vncol[1:128, NKV - 1, 0] = 0.0          # padded kv rows drop out of l
    ones65 = np.ones((65, 64), dtype=f32)
    zpad = np.zeros((64, PGH * PGW), dtype=f32)

    in_maps = []
    for c in range(8):
        b = c // 4
        h0 = (c % 4) * 3
        xt = np.zeros((768, NT), dtype=f32)
        xt[:, :N] = x[b].T
        wq = np.empty((3, 6, 128, 128), dtype=f32)
        wk = np.empty((3, 6, 128, 128), dtype=f32)
        for i in range(3):
            h = h0 + i
            wqT = qkv_w[h * DH:(h + 1) * DH, :].T            # [768, 64]
            wkT = qkv_w[D + h * DH: D + (h + 1) * DH, :].T
            wq[i] = np.concatenate([wqT, wqT], axis=1).reshape(6, 128, 128)
            wk[i] = np.concatenate([wkT, wkT], axis=1).reshape(6, 128, 128)
        wv = np.concatenate(
            [qkv_w[2 * D + (h0 + i) * DH: 2 * D + (h0 + i + 1) * DH, :].T
             for i in range(3)], axis=1).reshape(6, 128, 192)
        projt = np.ascontiguousarray(
            proj_w[:, h0 * DH: h0 * DH + 3 * DH].T.astype(f32))  # [192, 768]
        in_maps.append({
            "xt": np.ascontiguousarray(xt.reshape(6, 128, NT)),
            "wq": wq, "wk": wk,
            "wv": np.ascontiguousarray(wv),
            "projt": projt,
            "dconv": dconv, "convb": convb, "ident": ident,
            "vncol": vncol, "ones": ones65, "zpad": zpad,
        })
    return in_maps


def kernel(x, qkv_w, proj_w, proj_b, conv_w, conv_b):
    global LAST_RESULT
    from concourse.bass_utils import run_bass_kernel_spmd

    if "nc" not in _CACHE:
        _CACHE["nc"] = _build_nc()
    nc = _CACHE["nc"]

    in_maps = _prep_inputs(np.asarray(x, dtype=np.float32),
                           np.asarray(qkv_w, dtype=np.float32),
                           np.asarray(proj_w, dtype=np.float32),
                           np.asarray(conv_w, dtype=np.float32),
                           np.asarray(conv_b, dtype=np.float32))

    res = run_bass_kernel_spmd(nc, in_maps, core_ids=list(range(8)),
                               trace=TRACE)
    LAST_RESULT = res
    pb = np.asarray(proj_b, dtype=np.float32)
    out = np.zeros((B, N, D), dtype=np.float32)
    for c in range(8):
        out[c // 4] += res.results[c]["y"][:N, :]
    out += pb[None, None, :]
    return out
